# revision 1
# baseline (speedup 1.0000x reference)
"""Boundary-calculation module (4 fixed 3x3 Sobel-like kernels -> sqrt-sum-sq -> sigmoid)
as a Trainium2 Bass kernel, data-parallel over 8 NeuronCores (batch 32 -> 4 images/core).

Math: with integer-tap kernels E_k (reference kernels x4), the output is
    out = sigmoid(sqrt(SS)/8),  SS = E0^2 + E1^2 + E2^2 + E3^2
The filter bank is rotated into four cheap filters (exact identity):
    f0^2 + g2^2 = (2/3)*dv^2 + (1/3)*(diffv((3,4,3)h x))^2      [A, B]
    f1^2 + g3^2 = (2/3)*dh^2 + (1/3)*((3,4,3)v dh)^2            [D, C]
with dv/dh the vertical/horizontal central diffs (for any filter pairs u,v
with u u^T + v v^T fixed the sum of squares is invariant, and
(1,2,1)(1,2,1)^T + 2(1,1,1)(1,1,1)^T = (2/3) d d^T + (1/3)(3,4,3)(3,4,3)^T
where d = (0,1,0)).

Everything runs in bf16 (PSUM fp32): halves DMA traffic, gives 1-cycle/col
matmuls (vs 4 for fp32) and 2x DVE mode for the built-in adds. Engine split
per 103/104-row chunk, chosen to balance ACT/DVE/PE/Pool busy time:
  PE   : A (1 mm), B (3 mms, horizontal taps as shifted rhs), C (1 mm on
         dh), + SS = id*V + id*Q (2 accumulating id-matmuls, 2 of 3 chunks)
  DVE  : dh = xm - xp (2x), Q = (2/3)dh^2 + C^2 (custom, dh read straight
         from SBUF -- chunks are tile-aligned with a throwaway first row so
         no partition-offset read is needed), sigmoid as a degree-5 odd
         minimax poly (custom), SS-add for 1 of 3 chunks
  ACT  : U = Square([B|A] psum pair) -> bf16, t = Sqrt(SS/64); Square and
         Sqrt share one table set -> no table switches
  Pool : V = U.lo + U.hi (gpsimd add)
DMA handles the one-row partition offset on the way out.
"""

import os
import sys

sys.path.insert(0, "/opt/trn_rl_repo")

import numpy as np

import concourse.bacc as bacc
import concourse.bass as bass
import concourse.mybir as mybir
from concourse.tile import TileContext, add_dep_helper

AF = mybir.ActivationFunctionType
F32 = mybir.dt.float32
BF16 = mybir.dt.bfloat16

KERNEL_V = int(os.environ.get("KERNEL_V", "6"))


def _register_custom_ops():
    """Register custom DVE ops:
      SQUARE_ADD_ANT : out = in0^2 + in1
      SQ2S_ANT       : out = s0*in0^2 + in1^2
      SIGMOID_POLY_ANT: out = (((u*imm2+s1)*u+s0)*in0) + in1, u = in0^2
    """
    import concourse.dve_ops as dops
    from concourse.dve_spec import Spec, Src0, Src1, C0, C1, C2, lower, _has_src1
    from concourse.dve_uop import DveOpSpec

    if "SQ2S_ANT" in dops._SUB_OPCODE_FOR_NAME:
        return (
            dops._BY_NAME_ANT["SQ2S_ANT"],
            dops._BY_NAME_ANT["SQ2_ANT"],
            dops._BY_NAME_ANT["SQUARE_ADD_ANT"],
            dops._BY_NAME_ANT["SIGMOID_POLY_ANT"],
        )

    def make(name, row, spec):
        dops._SUB_OPCODE_FOR_NAME[name] = row
        shas = {}
        for ver in ("v3", "v4"):
            try:
                compiled = DveOpSpec(
                    name=name,
                    opcode=row,
                    uops=lower(spec, ver=ver),
                    rd1_en=_has_src1(spec),
                )
                shas[ver] = compiled.sha(ver)
            except Exception:
                pass
        op = dops.DveOp(name, spec, False, shas)
        dops.OPS.append(op)
        dops.CUSTOM_DVE_SPECS[name] = spec
        return op

    next_row = max(dops._SUB_OPCODE_FOR_NAME.values()) + 1
    # NB: sq(Src0) lowers to something the DVE firmware rejects
    # (NRT_EXEC_UNIT_UNRECOVERABLE on HW); Src0*Src0 works.
    sq2s_op = make(
        "SQ2S_ANT",
        next_row,
        Spec(
            body=Src0 * Src0 * C0 + Src1 * Src1,
            reference=lambda in0, in1, s0, s1, imm2: (
                in0.astype(np.float32) ** 2 * s0 + in1.astype(np.float32) ** 2
            ).astype(np.float32),
        ),
    )
    sqa_op = make(
        "SQUARE_ADD_ANT",
        next_row + 3,
        Spec(
            body=Src0 * Src0 + Src1,
            reference=lambda in0, in1, s0, s1, imm2: (
                in0.astype(np.float32) ** 2 + in1
            ).astype(np.float32),
        ),
    )
    sq2_op = make(
        "SQ2_ANT",
        next_row + 1,
        Spec(
            body=Src0 * Src0 + Src1 * Src1,
            reference=lambda in0, in1, s0, s1, imm2: (
                in0.astype(np.float32) ** 2 + in1.astype(np.float32) ** 2
            ).astype(np.float32),
        ),
    )
    u_node = Src0 * Src0
    sig_op = make(
        "SIGMOID_POLY_ANT",
        next_row + 2,
        Spec(
            body=(((u_node * C2 + C1) * u_node + C0) * Src0) + Src1,
            reference=lambda in0, in1, s0, s1, imm2: (
                ((in0.astype(np.float32) ** 2 * imm2 + s1) * in0**2 + s0) * in0 + in1
            ).astype(np.float32),
        ),
    )
    dops._BY_NAME_ANT = {
        "SQ2S_ANT": sq2s_op,
        "SQ2_ANT": sq2_op,
        "SQUARE_ADD_ANT": sqa_op,
        "SIGMOID_POLY_ANT": sig_op,
    }
    return sq2s_op, sq2_op, sqa_op, sig_op


B, H, W = 32, 512, 512
NCORES = 8
BPC = B // NCORES  # images per core

ALPHA = float(np.sqrt(2.0 / 3.0))  # weight of the central-diff maps
BETA = float(1.0 / np.sqrt(3.0))  # weight of the (3,4,3) maps

# Chunks are tile-aligned: compute partition m corresponds to image row
# t0+m, and for every chunk except the first the first computed row (m=0)
# is a throwaway duplicate so that dh (SBUF, partition-0-aligned) can feed
# the DVE square directly (DVE ops may read at most one PSUM operand and
# cannot read SBUF at a non-32-aligned partition offset; DMA-out can).
# (t0, Mc, K, vlo, nv): tile rows [t0, t0+K), compute rows m in [0, Mc),
# valid output rows m in [vlo, vlo+nv) -> image rows [t0+vlo, t0+vlo+nv).
CHUNKS = [
    (0, 103, 104, 0, 103),
    (102, 104, 105, 1, 103),
    (205, 104, 105, 1, 103),
    (308, 104, 105, 1, 103),
    (411, 101, 101, 1, 100),
]

# vertical tap sets (t=0 <-> dr=-1): out[m] = sum_t taps[t] * tile[m + t - 1]
TAPS = {
    "d3b": (3 * BETA, 0.0, -3 * BETA),  # B side columns (xm, xp)
    "d4b": (4 * BETA, 0.0, -4 * BETA),  # B center column (xc)
    "da": (ALPHA, 0.0, -ALPHA),  # A map (xc)
    "c343": (3 * BETA, 4 * BETA, 3 * BETA),  # C map (on dh)
}

# sigmoid(t) ~= 0.5 + t*(P_C1 + u*(P_C3 + u*P_C5)), u = t^2, t in [0, 1.04];
# minimax fit, max abs err 3.3e-6.
P_C1, P_C3, P_C5 = 0.24997775, -0.02066035, 0.0017408


def _band(K, M, taps):
    """Banded lhsT [K, M]: out[m] = sum_k V[k, m] * tile[k] (tile-aligned)."""
    V = np.zeros((K, M), np.float32)
    for m in range(M):
        for t in range(3):
            k = m + t - 1
            if 0 <= k < K:
                V[k, m] = taps[t]
    return V


def _build_weights():
    """Pack band matrices + a 128x128 identity into one [128, cols] bf16 array.

    Returns (wts, offmap, id_off) with offmap[(align, K, M)][tap_name] = col offset.
    """
    offmap = {}
    mats = []
    off = 0
    for t0, Mc, K, vlo, nv in CHUNKS:
        key = (K, Mc)
        if key in offmap:
            continue
        offmap[key] = {}
        for tn, taps in TAPS.items():
            offmap[key][tn] = off
            mats.append((off, _band(K, Mc, taps)))
            off += Mc
    id_off = off
    mats.append((off, np.eye(128, dtype=np.float32)))
    off += 128
    wts = np.zeros((128, off), np.float32)
    for o, V in mats:
        wts[: V.shape[0], o : o + V.shape[1]] = V
    return wts, offmap, id_off


SS_PE_PERIOD = int(os.environ.get("KERNEL_SS_PE_PERIOD", "3"))
# per-chunk V/SS engine pattern: e=all-PE id-matmuls, p=Pool V + PE ss, d=Pool V + DVE ss
V_MODE = os.environ.get("KERNEL_V_MODE", "sewe")
# per-chunk dh engine: d=DVE (2x), p=Pool
DH_MODE = os.environ.get("KERNEL_DH_MODE", "dpp")


def _build_nc(wts_cols, offmap, id_off, repeat=1):
    sq2s_op, sq2_op, sqa_op, sig_op = _register_custom_ops()
    nc = bacc.Bacc()
    x = nc.dram_tensor("x", [BPC, H, W], BF16, kind="ExternalInput")
    wt = nc.dram_tensor("wts", [128, wts_cols], BF16, kind="ExternalInput")
    y = nc.dram_tensor("y", [BPC, H, W], BF16, kind="ExternalOutput")

    with TileContext(nc) as tc:
        with (
            tc.tile_pool(name="wpool", bufs=1) as wpool,
            tc.tile_pool(name="dpool", bufs=int(os.environ.get("KERNEL_BUFS", "3"))) as dpool,
            tc.tile_pool(name="upool", bufs=int(os.environ.get("KERNEL_BUFS", "3"))) as upool,
            tc.tile_pool(name="rpool", bufs=int(os.environ.get("KERNEL_BUFS", "3"))) as rpool,
            tc.tile_pool(name="psum", bufs=2, space="PSUM") as psp,
        ):
            # Explicit x-tile ring: pad columns memset ONCE (outside the
            # chunk loop) so per-chunk DVE work is only dh + sq2s + sigmoid.
            NXT = int(os.environ.get("KERNEL_NXT", "5"))
            xts = []
            for i in range(NXT):
                t = wpool.tile([128, 516], BF16, tag=f"xt{i}")
                # pad memsets on the otherwise-idle gpsimd: keeps the DVE
                # queue free for the first dh
                nc.gpsimd.memset(t[:, 1:2], 0.0)
                nc.gpsimd.memset(t[:, 514:515], 0.0)
                xts.append(t)

            # Weight tiles: either one DMA for everything (fewer HWDGE
            # slots before the first x tiles) or per-group DMAs.
            group_keys = list(offmap.keys())  # insertion order: top first
            gsizes = {k: len(TAPS) * k[1] for k in group_keys}
            gstart = {}
            off = 0
            for k in group_keys:
                gstart[k] = off
                off += gsizes[k]
            assert off == id_off
            wtiles = {}
            last_k = group_keys[-1]
            if os.environ.get("KERNEL_WSPLIT", "1") == "1":
                for gi, k in enumerate(group_keys):
                    cols = gsizes[k] + (128 if k == last_k else 0)
                    gt = wpool.tile([128, cols], BF16, tag=f"wg{gi}")
                    nc.sync.dma_start(
                        out=gt[:], in_=wt[:, gstart[k] : gstart[k] + cols]
                    )
                    wtiles[k] = gt
                wid = wtiles[last_k][:, gsizes[last_k] : gsizes[last_k] + 128]
            else:
                wall = wpool.tile([128, wts_cols], BF16, tag="wall")
                nc.sync.dma_start(out=wall[:], in_=wt[:])
                for k in group_keys:
                    wtiles[k] = wall[:, gstart[k] : gstart[k] + gsizes[k]]
                wid = wall[:, id_off : id_off + 128]
            halfs = wpool.tile([128, 1024], BF16, tag="halfs")
            nc.vector.memset(halfs[:], 0.5)

            def issue_in_dma(idx, img, t0, K):
                xt = xts[idx % NXT]
                nc.sync.dma_start(out=xt[:K, 2:514], in_=x[img, t0 : t0 + K, :])

            def front(idx, img, t0, Mc, K):
                voff = offmap[(K, Mc)]
                xt = xts[idx % NXT]

                xm = xt[:K, 1:513]
                xc = xt[:K, 2:514]
                xp = xt[:K, 3:515]

                gkey = (K, Mc)
                gt = wtiles[gkey]

                def wv(tn):
                    o = voff[tn] - gstart[gkey]
                    return gt[0:K, o : o + Mc]

                # dh = x[c-1] - x[c+1]; feeds the C band (PE) and the DVE
                # square directly (partition m of dh = compute row m)
                dh = dpool.tile([128, 512], BF16, tag="dh")
                if DH_MODE[idx % len(DH_MODE)] == "p":
                    nc.gpsimd.tensor_sub(out=dh[:K], in0=xm, in1=xp)
                else:
                    nc.vector.tensor_sub(out=dh[:K], in0=xm, in1=xp)

                ps1 = psp.tile([128, 1024], F32, tag="ba")  # [B | A]
                ps2 = psp.tile([128, 512], F32, tag="c")  # C

                # x-direct MMs first: they only need the xt DMA, while the
                # C MM also needs dh (DVE); emitting C last keeps the
                # in-order PE queue from stalling on the dh chain.
                nc.tensor.matmul(ps1[:Mc, 0:512], wv("d4b"), xc, start=True, stop=False)
                nc.tensor.matmul(ps1[:Mc, 0:512], wv("d3b"), xm, start=False, stop=False)
                nc.tensor.matmul(ps1[:Mc, 0:512], wv("d3b"), xp, start=False, stop=True)
                nc.tensor.matmul(
                    ps1[:Mc, 512:1024], wv("da"), xc, start=True, stop=True
                )
                nc.tensor.matmul(ps2[:Mc, :], wv("c343"), dh[:K], start=True, stop=True)
                return ps1, ps2, dh

            def back_a(st):
                """Squares and adds; SS ends in PSUM (own bank) or SBUF ssb."""
                (ps1, ps2, dh), (img, t0, Mc, vlo, nv, cidx) = st
                M = Mc
                mode = V_MODE[cidx % len(V_MODE)]
                u = upool.tile([128, 1024], BF16, tag="u")
                # Q = (2/3) dh^2 + C^2 (DVE custom; one PSUM operand)
                q = upool.tile([128, 512], BF16, tag="q")
                nc.vector._custom_dve(
                    sq2s_op,
                    out=q[:M],
                    in0=dh[:M, :],
                    in1=ps2[:M, :],
                    s0=ALPHA * ALPHA,
                )
                if mode == "s":
                    # split-square: ACT squares only B; DVE folds A^2 into q
                    nc.scalar.activation(u[:M, 0:512], ps1[:M, 0:512], AF.Square)
                    q2 = upool.tile([128, 512], BF16, tag="q2")
                    nc.vector._custom_dve(
                        sqa_op, out=q2[:M], in0=ps1[:M, 512:1024], in1=q[:M]
                    )
                    ss = psp.tile([128, 512], F32, tag="ss")
                    nc.tensor.matmul(
                        ss[:M, :], wid[0:M, 0:M], u[:M, 0:512],
                        start=True, stop=False,
                    )
                    nc.tensor.matmul(
                        ss[:M, :], wid[0:M, 0:M], q2[:M, :], start=False, stop=True
                    )
                    return ss[:M, :]
                # U = [B^2 | A^2] (ACT, Square in the sqrt table set)
                nc.scalar.activation(u[:M], ps1[:M, :], AF.Square)
                if mode == "e":
                    # SS = B^2 + A^2 + Q entirely via PE id-matmuls (no Pool
                    # hop in the chain)
                    ss = psp.tile([128, 512], F32, tag="ss")
                    nc.tensor.matmul(
                        ss[:M, :], wid[0:M, 0:M], u[:M, 0:512], start=True, stop=False
                    )
                    nc.tensor.matmul(
                        ss[:M, :], wid[0:M, 0:M], u[:M, 512:1024],
                        start=False, stop=False,
                    )
                    nc.tensor.matmul(
                        ss[:M, :], wid[0:M, 0:M], q[:M, :], start=False, stop=True
                    )
                    return ss[:M, :]
                # V = A^2 + B^2 (gpsimd or DVE 2x)
                v = upool.tile([128, 512], BF16, tag="v")
                if mode in ("w", "t"):
                    nc.vector.tensor_add(
                        out=v[:M], in0=u[:M, 0:512], in1=u[:M, 512:1024]
                    )
                else:
                    nc.gpsimd.tensor_add(
                        out=v[:M], in0=u[:M, 0:512], in1=u[:M, 512:1024]
                    )
                if mode == "t":
                    ssb = upool.tile([128, 512], BF16, tag="ssb")
                    nc.vector.tensor_add(out=ssb[:M], in0=v[:M], in1=q[:M])
                    return ssb[:M]
                if mode in ("p", "w"):
                    # SS = V + Q via identity matmuls into a dedicated bank
                    ss = psp.tile([128, 512], F32, tag="ss")
                    nc.tensor.matmul(
                        ss[:M, :], wid[0:M, 0:M], v[:M, :], start=True, stop=False
                    )
                    nc.tensor.matmul(
                        ss[:M, :], wid[0:M, 0:M], q[:M, :], start=False, stop=True
                    )
                    ss_ap = ss[:M, :]
                else:  # "d"
                    ssb = upool.tile([128, 512], BF16, tag="ssb")
                    nc.vector.tensor_add(out=ssb[:M], in0=v[:M], in1=q[:M])
                    ss_ap = ssb[:M]
                return ss_ap

            def back_b(ss_ap, img, t0, Mc, vlo, nv, tail=False):
                """sqrt + sigmoid + store (valid rows only; DMA handles the
                partition offset that engines cannot). tail=True splits the
                work into column halves so the drain chain pipelines."""
                M = Mc
                rt = rpool.tile([128, 512], BF16, tag="rt")
                cols = ((0, 256), (256, 256)) if tail else ((0, 512),)
                for c0, cw in cols:
                    nc.scalar.activation(
                        rt[:M, c0 : c0 + cw],
                        ss_ap[:, c0 : c0 + cw],
                        AF.Sqrt,
                        scale=1.0 / 64.0,
                    )
                    nc.vector._custom_dve(
                        sig_op,
                        out=rt[:M, c0 : c0 + cw],
                        in0=rt[:M, c0 : c0 + cw],
                        in1=halfs[:M, c0 : c0 + cw],
                        s0=P_C1,
                        s1=P_C3,
                        imm2=P_C5,
                    )
                    nc.sync.dma_start(
                        out=y[img, t0 + vlo : t0 + vlo + nv, c0 : c0 + cw],
                        in_=rt[vlo : vlo + nv, c0 : c0 + cw],
                    )

            PREFETCH = int(os.environ.get("KERNEL_PREFETCH", "2"))
            for rep in range(repeat):
                chunk_list = []
                for img in range(BPC):
                    for t0, Mc, K, vlo, nv in CHUNKS:
                        chunk_list.append((img, t0, Mc, K, vlo, nv))
                if os.environ.get("KERNEL_CMAJOR") == "1":
                    # chunk-index-major order: all images' chunk 0 first, ...
                    chunk_list.sort(key=lambda c: (c[1], c[0]))
                n = len(chunk_list)
                pend_a = None
                pend_b = None
                for i in range(PREFETCH):
                    img, t0, Mc, K, vlo, nv = chunk_list[i]
                    issue_in_dma(i, img, t0, K)
                for i in range(n):
                    img, t0, Mc, K, vlo, nv = chunk_list[i]
                    if i + PREFETCH < n:
                        nxt = chunk_list[i + PREFETCH]
                        issue_in_dma(i + PREFETCH, nxt[0], nxt[1], nxt[3])
                    st = front(i, img, t0, Mc, K)
                    newb = None
                    if pend_a is not None:
                        ss_ap = back_a(pend_a)
                        meta = pend_a[1]
                        newb = (ss_ap, meta[0], meta[1], meta[2], meta[3], meta[4])
                    if pend_b is not None:
                        back_b(*pend_b)
                    pend_b = newb
                    pend_a = (st, (img, t0, Mc, vlo, nv, i))
                tail_split = os.environ.get("KERNEL_TAILSPLIT", "0") == "1"
                if pend_b is not None:
                    back_b(*pend_b, tail=tail_split)
                ss_ap = back_a(pend_a)
                meta = pend_a[1]
                back_b(
                    ss_ap, meta[0], meta[1], meta[2], meta[3], meta[4],
                    tail=tail_split,
                )

    nc.compile()
    return nc


_CACHE = {}


def _get_nc():
    global KERNEL_V
    KERNEL_V = int(os.environ.get("KERNEL_V", "6"))
    repeat = int(os.environ.get("KERNEL_REPEAT", "1"))
    key = ("nc", repeat, KERNEL_V)
    if key not in _CACHE:
        wts, offmap, id_off = _build_weights()
        _CACHE["wts"] = wts
        _CACHE[key] = _build_nc(wts.shape[1], offmap, id_off, repeat=repeat)
    return _CACHE[key], _CACHE["wts"]


_last_result = None


def kernel(pred_mask: np.ndarray) -> np.ndarray:
    global _last_result
    from concourse.bass_utils import run_bass_kernel_spmd

    assert pred_mask.shape == (B, 1, H, W), pred_mask.shape
    nc, wts = _get_nc()
    bf16 = mybir.dt.np(BF16)
    xs = np.ascontiguousarray(pred_mask.reshape(B, H, W)).astype(bf16)
    wts_b = wts.astype(bf16)
    in_maps = [
        {"x": xs[i * BPC : (i + 1) * BPC], "wts": wts_b} for i in range(NCORES)
    ]
    trace = bool(os.environ.get("KERNEL_TRACE"))
    res = run_bass_kernel_spmd(
        nc, in_maps, core_ids=list(range(NCORES)), trace=trace
    )
    _last_result = res
    out = np.stack([np.asarray(r["y"]) for r in res.results], axis=0)
    return out.reshape(B, 1, H, W).astype(np.float32)

